# revision 32
# baseline (speedup 1.0000x reference)
"""AdaptiveAttention Trainium2 kernel (8 NeuronCores, data-parallel over batch).

reference:
  Vt = swapaxes(V,1,2)                                  (B,P,H)
  img = einsum('bph,qh->bpq', Vt, Wv_w) + Wv_b          (B,P,P)
  htp = ht @ Wg_w.T                                     (B,P)
  z   = tanh(img + htp[:,:,None]) @ wh                  (B,P)
  alpha = softmax(z, axis=1)
  ctx = einsum('bph,bp->bh', Vt, alpha)                 (B,H)
  avs = tanh(st @ Ws_w.T + Ws_b + htp) @ wh             (B,)
  beta = softmax(concat[z, avs])[:, -1:]
  c_t = beta*st + (1-beta)*ctx
returns (c_t, ctx, alpha, beta)
"""

import numpy as np
import ml_dtypes

import concourse.bass as bass
import concourse.bacc as bacc
import concourse.tile as tile
from concourse import mybir
from concourse.bass_utils import run_bass_kernel_spmd
from contextlib import ExitStack

BF16 = ml_dtypes.bfloat16
FP = mybir.dt.float32
BF = mybir.dt.bfloat16

B, H, P = 4096, 1024, 49
NCORES = 8
BLOC = B // NCORES        # 512 batches per core
HALF = 64                 # batches per half-group (softmax granularity)
BT = 8                    # batches per b-tile (img matmul free dim = BT*P=392)
NPAIR = HALF // 2         # 32 pairs per half
FREE = BT * P             # 392
HC = 8                    # 128-row chunks of H
NCH = (HALF * P + 127) // 128   # 25: 128-row chunks of a half's (b,p) rows


def build_nc(nhalf=BLOC // HALF):
    bloc = nhalf * HALF
    nc = bacc.Bacc("TRN2", target_bir_lowering=False, debug=False, num_devices=NCORES)

    V_IMG = nc.declare_dram_parameter("v_img", [bloc // HALF, 2, 128, HC, HALF * P // 2], BF, False)
    V_CTX = nc.declare_dram_parameter("v_ctx", [HC, nhalf, 128, NCH, 128], BF, False)
    EPRM = nc.declare_dram_parameter("eperm", [P, NCH, 128], BF, False)
    BMSK = nc.declare_dram_parameter("bmask", [128, NCH, HALF], BF, False)
    HTT = nc.declare_dram_parameter("htT", [128, HC, bloc], BF, False)
    STT = nc.declare_dram_parameter("stT", [128, HC, bloc], BF, False)
    STN = nc.declare_dram_parameter("stn", [bloc, H], FP, False)
    WVT = nc.declare_dram_parameter("wvT", [128, HC, P], BF, False)
    WGT = nc.declare_dram_parameter("wgT", [128, HC, P], BF, False)
    WST = nc.declare_dram_parameter("wsT", [128, HC, P], BF, False)
    WHT = nc.declare_dram_parameter("whT", [P, 1], BF, False)
    WVB = nc.declare_dram_parameter("wv_b", [P, 1], FP, False)
    WSB = nc.declare_dram_parameter("ws_b", [P, 1], FP, False)
    ONES = nc.declare_dram_parameter("ones49", [1, P], BF, False)
    EYE = nc.declare_dram_parameter("eye128", [128, 128], FP, False)
    OUT = nc.declare_dram_parameter("out", [bloc, 2 * H + P + 1], FP, True)

    with tile.TileContext(nc) as tc, ExitStack() as ctx:
        singles = ctx.enter_context(tc.tile_pool(name="singles", bufs=1))

        wvT = singles.tile([128, HC, P], BF, tag="wvT")
        nc.sync.dma_start(out=wvT, in_=WVT[:, :, :])
        wgT = singles.tile([128, HC, P], BF, tag="wgT")
        nc.sync.dma_start(out=wgT, in_=WGT[:, :, :])
        wsT = singles.tile([128, HC, P], BF, tag="wsT")
        nc.sync.dma_start(out=wsT, in_=WST[:, :, :])
        whT = singles.tile([P, 1], BF, tag="whT")
        nc.sync.dma_start(out=whT, in_=WHT[:, :])
        wvb = singles.tile([P, 1], FP, tag="wvb")
        nc.sync.dma_start(out=wvb, in_=WVB[:, :])
        wsb = singles.tile([P, 1], FP, tag="wsb")
        nc.sync.dma_start(out=wsb, in_=WSB[:, :])
        ones = singles.tile([1, P], BF, tag="ones")
        nc.sync.dma_start(out=ones, in_=ONES[:, :])
        eye = singles.tile([128, 128], FP, tag="eye")
        nc.sync.dma_start(out=eye, in_=EYE[:, :])
        eprm = singles.tile([P, NCH, 128], BF, tag="eprm")
        nc.sync.dma_start(out=eprm, in_=EPRM[:, :, :])
        bmsk = singles.tile([128, NCH, HALF], BF, tag="bmsk")
        nc.sync.dma_start(out=bmsk, in_=BMSK[:, :, :])

        avs_row = singles.tile([1, bloc], FP, tag="avs")

        # ---- phase A: htp/stp/avs, htp rows in (b,p) order ----
        with (
            tc.tile_pool(name="pa_sb", bufs=1) as pa,
            tc.tile_pool(name="pa_ps", bufs=2, space="PSUM") as pap,
        ):
            htT = pa.tile([128, HC, bloc], BF, tag="htT")
            nc.sync.dma_start(out=htT, in_=HTT[:, :, :])
            stT = pa.tile([128, HC, bloc], BF, tag="stT")
            nc.sync.dma_start(out=stT, in_=STT[:, :, :])

            ps_htp = pap.tile([P, bloc], FP, tag="p")
            for hc in range(HC):
                nc.tensor.matmul(ps_htp, wgT[:, hc, :], htT[:, hc, :],
                                 start=(hc == 0), stop=(hc == HC - 1))
            htp = pa.tile([P, bloc], FP, tag="htp")
            nc.vector.tensor_copy(out=htp, in_=ps_htp)

            ps_stp = pap.tile([P, bloc], FP, tag="p")
            for hc in range(HC):
                nc.tensor.matmul(ps_stp, wsT[:, hc, :], stT[:, hc, :],
                                 start=(hc == 0), stop=(hc == HC - 1))
            sh = pa.tile([P, bloc], FP, tag="sh")
            nc.vector.tensor_add(out=sh, in0=htp, in1=ps_stp)
            avt = pa.tile([P, bloc], BF, tag="avt")
            nc.scalar.activation(out=avt, in_=sh,
                                 func=mybir.ActivationFunctionType.Tanh,
                                 bias=wsb, scale=1.0)
            ps_avs = pap.tile([1, bloc], FP, tag="pavs")
            nc.tensor.matmul(ps_avs, whT, avt, start=True, stop=True)
            nc.vector.tensor_copy(out=avs_row, in_=ps_avs)

            # htpT (b-major, bf16) kept resident for per-half row extraction
            g128 = min(128, bloc)
            ngr = bloc // g128
            htpT = singles.tile([g128, ngr, P], BF, tag="htpT")
            for g in range(ngr):
                ps_t = pap.tile([g128, P], FP, tag="pt")
                nc.tensor.transpose(ps_t, htp[:, g * g128:(g + 1) * g128],
                                    eye[0:P, 0:P])
                nc.vector.tensor_copy(out=htpT[:, g, :], in_=ps_t)

        # ---- main: fine-grained software pipeline over halves ----
        # per half i: img/tanh/z btiles (pass1) run while the previous
        # half's softmax->S->context matmuls (pass2) interleave btile-wise.
        vimg = ctx.enter_context(tc.tile_pool(name="vimg", bufs=3))
        vctxp = ctx.enter_context(tc.tile_pool(name="vctx", bufs=7))
        thp = ctx.enter_context(tc.tile_pool(name="th", bufs=3))
        smp = ctx.enter_context(tc.tile_pool(name="sm", bufs=2))
        outp = ctx.enter_context(tc.tile_pool(name="outp", bufs=2))
        pimg = ctx.enter_context(tc.tile_pool(name="pimg", bufs=2, space="PSUM"))
        pz = ctx.enter_context(tc.tile_pool(name="pz", bufs=2, space="PSUM"))
        pt = ctx.enter_context(tc.tile_pool(name="pt", bufs=2, space="PSUM"))
        pctx = ctx.enter_context(tc.tile_pool(name="pctx", bufs=2, space="PSUM"))

        g128 = min(128, bloc)
        HB = HALF * P // 2          # free size of one A/B img tile (4 btiles)
        vi_tiles = {}               # (i, 0|1) -> tile
        vc_tiles = {}               # (i, hc) -> tile
        sm_st = {}                  # i -> dict of softmax tiles

        def emit_vi(i, j):
            vt = vimg.tile([128, HC, HB], BF, tag="vi", name=f"vi{i}_{j}")
            nc.sync.dma_start(out=vt, in_=V_IMG[i, j])
            vi_tiles[(i, j)] = vt

        def emit_rows(i):
            rows_h = smp.tile([1, HALF * P], BF, tag="rows")
            pb = (i * HALF) % g128
            nc.gpsimd.dma_start(
                out=rows_h.rearrange("o (b q) -> o b q", b=HALF, q=P),
                in_=htpT[pb:pb + HALF, (i * HALF) // g128, :])
            sm_st[i] = {"rows": rows_h}

        def emit_vc(i, hc):
            vc = vctxp.tile([128, NCH, 128], BF, tag="vc",
                            name=f"vc{i}_{hc}")
            nc.sync.dma_start(out=vc, in_=V_CTX[hc, i])
            vc_tiles[(i, hc)] = vc

        def open_z50(i):
            z50 = smp.tile([HALF, P + 1], FP, tag="z50")
            nc.gpsimd.dma_start(out=z50[:, P:P + 1],
                                in_=avs_row[0:1, i * HALF:(i + 1) * HALF])
            sm_st[i]["z50"] = z50

        def img_btile(i, t):
            st = sm_st[i]
            vt = vi_tiles[(i, t // 4)]
            tf = (t % 4) * FREE
            ps_img = pimg.tile([P, FREE], FP, tag="img")
            for hc in range(HC):
                nc.tensor.matmul(ps_img, wvT[:, hc, :],
                                 vt[:, hc, tf:tf + FREE],
                                 start=(hc == 0), stop=False)
            nc.tensor.matmul(ps_img, ones,
                             st["rows"][0:1, t * FREE:(t + 1) * FREE],
                             start=False, stop=True)
            th = thp.tile([P, FREE], BF, tag="th")
            nc.scalar.activation(out=th, in_=ps_img,
                                 func=mybir.ActivationFunctionType.Tanh,
                                 bias=wvb, scale=1.0)
            ps_z = pz.tile([1, FREE], FP, tag="z")
            nc.tensor.matmul(ps_z, whT, th, start=True, stop=True)
            zrow = thp.tile([1, FREE], FP, tag="zrow")
            nc.vector.tensor_copy(out=zrow, in_=ps_z)
            nc.gpsimd.dma_start(
                out=st["z50"][t * BT:(t + 1) * BT, 0:P],
                in_=zrow.rearrange("o (bi p) -> o bi p", bi=BT, p=P))
            if t == 3:
                vi_tiles.pop((i, 0))
            elif t == 7:
                vi_tiles.pop((i, 1))

        def softmax_dve(i):
            st = sm_st[i]
            z50 = st["z50"]
            m = smp.tile([HALF, 1], FP, tag="m")
            nc.vector.reduce_max(out=m, in_=z50, axis=mybir.AxisListType.X)
            mneg = smp.tile([HALF, 1], FP, tag="mneg")
            nc.scalar.mul(out=mneg, in_=m, mul=-1.0)
            e = smp.tile([HALF, P + 1], FP, tag="e")
            nc.scalar.activation(out=e, in_=z50,
                                 func=mybir.ActivationFunctionType.Exp,
                                 bias=mneg, scale=1.0)
            s49 = smp.tile([HALF, 1], FP, tag="s49")
            nc.vector.reduce_sum(out=s49, in_=e[:, 0:P],
                                 axis=mybir.AxisListType.X)
            s50 = smp.tile([HALF, 1], FP, tag="s50")
            nc.vector.tensor_add(out=s50, in0=s49, in1=e[:, P:P + 1])
            nc.vector.reciprocal(out=s49, in_=s49)
            nc.vector.reciprocal(out=s50, in_=s50)
            alpha = smp.tile([HALF, P], FP, tag="alpha")
            nc.vector.tensor_scalar_mul(out=alpha, in0=e[:, 0:P], scalar1=s49)
            beta = smp.tile([HALF, 1], FP, tag="beta")
            nc.vector.tensor_mul(out=beta, in0=e[:, P:P + 1], in1=s50)
            st["alpha"], st["beta"] = alpha, beta

        def emit_aT_S(i):
            st = sm_st[i]
            ps_at = pt.tile([P, HALF], FP, tag="pt")
            nc.tensor.transpose(ps_at, st["alpha"], eye[0:HALF, 0:HALF])
            alphaT = smp.tile([P, HALF], BF, tag="alphaT")
            nc.vector.tensor_copy(out=alphaT, in_=ps_at)
            D = smp.tile([128, NCH, HALF], BF, tag="D")
            for c in range(NCH):
                ps_d = pt.tile([128, HALF], FP, tag="pt", name="psd")
                nc.tensor.matmul(ps_d, eprm[:, c, :], alphaT,
                                 start=True, stop=True)
                nc.vector.tensor_mul(out=D[:, c, :], in0=ps_d,
                                     in1=bmsk[:, c, :])
            st["D"] = D
            cc = outp.tile([HALF, 2 * H], FP, tag="cc", name="cc")
            st["cc"] = cc
            st["ctx_sb"] = cc[:, H:2 * H]

        def ctx_hc(i, hc):
            st = sm_st[i]
            D = st["D"]
            vc = vc_tiles.pop((i, hc))
            ps_hc = pctx.tile([128, HALF], FP, tag="ctx")
            for c in range(NCH):
                nc.tensor.matmul(ps_hc, vc[:, c, :], D[:, c, :],
                                 start=(c == 0), stop=(c == NCH - 1))
            ctxh = thp.tile([128, HALF], FP, tag="ctxh")
            nc.vector.tensor_copy(out=ctxh, in_=ps_hc)
            ps_ct = pt.tile([HALF, 128], FP, tag="pt")
            nc.tensor.transpose(ps_ct, ctxh, eye)
            nc.vector.tensor_copy(
                out=st["ctx_sb"][:, hc * 128:(hc + 1) * 128], in_=ps_ct)

        def blend_out(i):
            st = sm_st.pop(i)
            ctx_sb, alpha, beta = st["ctx_sb"], st["alpha"], st["beta"]
            stn = outp.tile([HALF, H], FP, tag="stn")
            nc.gpsimd.dma_start(out=stn, in_=STN[i * HALF:(i + 1) * HALF, :])
            nc.vector.tensor_sub(out=stn, in0=stn, in1=ctx_sb)
            nc.vector.tensor_scalar_mul(out=stn, in0=stn, scalar1=beta)
            cc = st["cc"]
            nc.vector.tensor_add(out=cc[:, 0:H], in0=stn, in1=ctx_sb)
            sl = slice(i * HALF, (i + 1) * HALF)
            nc.gpsimd.dma_start(out=OUT[sl, 0:2 * H], in_=cc)
            nc.gpsimd.dma_start(out=OUT[sl, 2 * H:2 * H + P], in_=alpha)
            nc.gpsimd.dma_start(out=OUT[sl, 2 * H + P:2 * H + P + 1],
                                in_=beta)

        emit_vi(0, 0)
        emit_vi(0, 1)
        emit_rows(0)
        for i in range(nhalf):
            open_z50(i)
            if i > 0:
                softmax_dve(i - 1)
                emit_vc(i - 1, 0)
                emit_vc(i - 1, 1)
            if i + 1 < nhalf:
                emit_vi(i + 1, 0)
                emit_rows(i + 1)
            for t in range(BT):
                if i > 0 and t + 2 < HC:
                    emit_vc(i - 1, t + 2)
                if t == 4 and i + 1 < nhalf:
                    emit_vi(i + 1, 1)
                img_btile(i, t)
                if i > 0 and t == 0:
                    emit_aT_S(i - 1)
                if i > 0 and t >= 1:
                    ctx_hc(i - 1, t - 1)
            if i > 0:
                ctx_hc(i - 1, HC - 1)
                blend_out(i - 1)
        # tail: last half's softmax + context
        last = nhalf - 1
        softmax_dve(last)
        emit_aT_S(last)
        for hc in range(HC):
            emit_vc(last, hc)
            ctx_hc(last, hc)
        blend_out(last)

    nc.finalize()
    return nc


def prep_inputs_core(Vc, htc, stc, w):
    """Per-core input map. Vc (bloc,H,P) f32, htc/stc (bloc,H) f32."""
    bloc = Vc.shape[0]
    nhalf = bloc // HALF
    # v_img[i, j, h, hc, (bq p)] = V[i*64+j*32+bq, hc*128+h, p]
    v_img = np.ascontiguousarray(
        Vc.reshape(nhalf, 2, 32, HC, 128, P).transpose(0, 1, 4, 3, 2, 5)
        .reshape(nhalf, 2, 128, HC, HALF * P // 2)
    ).astype(BF16)
    # v_ctx[hc, half, c, r, h] = V[half*64 + (128c+r)//49, hc*128+h, (128c+r)%49]
    vt = (Vc.reshape(nhalf, HALF, HC, 128, P).transpose(2, 0, 1, 4, 3)
          .reshape(HC, nhalf, HALF * P, 128))
    v_ctx = np.zeros((HC, nhalf, NCH * 128, 128), np.float32)
    v_ctx[:, :, :HALF * P, :] = vt
    v_ctx = np.ascontiguousarray(
        v_ctx.reshape(HC, nhalf, NCH, 128, 128).transpose(0, 1, 3, 2, 4)
    ).astype(BF16)
    htT = np.ascontiguousarray(
        htc.reshape(bloc, HC, 128).transpose(2, 1, 0)).astype(BF16)
    stT = np.ascontiguousarray(
        stc.reshape(bloc, HC, 128).transpose(2, 1, 0)).astype(BF16)
    m = {
        "v_img": v_img, "v_ctx": v_ctx, "htT": htT, "stT": stT,
        "stn": np.ascontiguousarray(stc, dtype=np.float32),
    }
    m.update(w)
    return m


def prep_weights(Wv_w, Wv_b, Ws_w, Ws_b, Wg_w, wh_w):
    def wT(W):  # (P, H) -> (128, HC, P)
        return np.ascontiguousarray(
            W.reshape(P, HC, 128).transpose(2, 1, 0)).astype(BF16)
    return {
        "wvT": wT(Wv_w), "wgT": wT(Wg_w), "wsT": wT(Ws_w),
        "whT": np.ascontiguousarray(wh_w.reshape(P, 1)).astype(BF16),
        "wv_b": np.ascontiguousarray(Wv_b.reshape(P, 1), dtype=np.float32),
        "ws_b": np.ascontiguousarray(Ws_b.reshape(P, 1), dtype=np.float32),
        "ones49": np.ones((1, P), dtype=BF16),
        "eye128": np.eye(128, dtype=np.float32),
        "eperm": _eperm(), "bmask": _bmask(),
    }


def _eperm():
    e = np.zeros((P, NCH, 128), np.float32)
    r = np.arange(NCH * 128)
    valid = r < HALF * P
    e[(r % P)[valid], (r // 128)[valid], (r % 128)[valid]] = 1.0
    return e.astype(BF16)


def _bmask():
    b = np.zeros((128, NCH, HALF), np.float32)
    r = np.arange(NCH * 128)
    valid = r < HALF * P
    b[(r % 128)[valid], (r // 128)[valid], (r // P)[valid]] = 1.0
    return b.astype(BF16)


_nc_cache = {}


def _get_nc(nhalf):
    if nhalf not in _nc_cache:
        _nc_cache[nhalf] = build_nc(nhalf)
    return _nc_cache[nhalf]


def kernel(V, ht, st, Wv_w, Wv_b, Ws_w, Ws_b, Wg_w, wh_w, **_run_kw):
    V = np.asarray(V, dtype=np.float32)
    ht = np.asarray(ht, dtype=np.float32)
    st = np.asarray(st, dtype=np.float32)
    w = prep_weights(np.asarray(Wv_w, np.float32), np.asarray(Wv_b, np.float32),
                     np.asarray(Ws_w, np.float32), np.asarray(Ws_b, np.float32),
                     np.asarray(Wg_w, np.float32), np.asarray(wh_w, np.float32))
    nc = _get_nc(BLOC // HALF)
    in_maps = []
    for c in range(NCORES):
        sl = slice(c * BLOC, (c + 1) * BLOC)
        in_maps.append(prep_inputs_core(V[sl], ht[sl], st[sl], w))
    res = run_bass_kernel_spmd(nc, in_maps, core_ids=list(range(NCORES)), **_run_kw)
    outs = [res.results[c]["out"] for c in range(NCORES)]
    full = np.concatenate(outs, axis=0)
    c_t = full[:, 0:H]
    ctxo = full[:, H:2 * H]
    alpha = full[:, 2 * H:2 * H + P]
    beta = full[:, 2 * H + P:2 * H + P + 1]
    kernel.last_result = res
    return (c_t, ctxo, alpha, beta)


# revision 33
# speedup vs baseline: 1.1964x; 1.1964x over previous
"""AdaptiveAttention Trainium2 kernel (8 NeuronCores, data-parallel over batch).

reference:
  Vt = swapaxes(V,1,2)                                  (B,P,H)
  img = einsum('bph,qh->bpq', Vt, Wv_w) + Wv_b          (B,P,P)
  htp = ht @ Wg_w.T                                     (B,P)
  z   = tanh(img + htp[:,:,None]) @ wh                  (B,P)
  alpha = softmax(z, axis=1)
  ctx = einsum('bph,bp->bh', Vt, alpha)                 (B,H)
  avs = tanh(st @ Ws_w.T + Ws_b + htp) @ wh             (B,)
  beta = softmax(concat[z, avs])[:, -1:]
  c_t = beta*st + (1-beta)*ctx
returns (c_t, ctx, alpha, beta)
"""

import numpy as np
import ml_dtypes

import concourse.bass as bass
import concourse.bacc as bacc
import concourse.tile as tile
from concourse import mybir
from concourse.bass_utils import run_bass_kernel_spmd
from contextlib import ExitStack

BF16 = ml_dtypes.bfloat16
FP = mybir.dt.float32
BF = mybir.dt.bfloat16

B, H, P = 4096, 1024, 49
NCORES = 8
BLOC = B // NCORES        # 512 batches per core
HALF = 64                 # batches per half-group (softmax granularity)
BT = 8                    # batches per b-tile (img matmul free dim = BT*P=392)
NPAIR = HALF // 2         # 32 pairs per half
FREE = BT * P             # 392
HC = 8                    # 128-row chunks of H
NCH = (HALF * P + 127) // 128   # 25: 128-row chunks of a half's (b,p) rows


def build_nc(nhalf=BLOC // HALF):
    bloc = nhalf * HALF
    nc = bacc.Bacc("TRN2", target_bir_lowering=False, debug=False, num_devices=NCORES)

    V_IMG = nc.declare_dram_parameter("v_img", [bloc // HALF, 2, 128, HC, HALF * P // 2], BF, False)
    V_CTX = nc.declare_dram_parameter("v_ctx", [HC, nhalf, 128, NCH, 128], BF, False)
    EPRM = nc.declare_dram_parameter("eperm", [P, NCH, 128], BF, False)
    BMSK = nc.declare_dram_parameter("bmask", [128, NCH, HALF], BF, False)
    HTT = nc.declare_dram_parameter("htT", [128, HC, bloc], BF, False)
    STT = nc.declare_dram_parameter("stT", [128, HC, bloc], BF, False)
    STN = nc.declare_dram_parameter("stn", [bloc, H], FP, False)
    WVT = nc.declare_dram_parameter("wvT", [128, HC, P], BF, False)
    WGT = nc.declare_dram_parameter("wgT", [128, HC, P], BF, False)
    WST = nc.declare_dram_parameter("wsT", [128, HC, P], BF, False)
    WHT = nc.declare_dram_parameter("whT", [P, 1], BF, False)
    WVB = nc.declare_dram_parameter("wv_b", [P, 1], FP, False)
    WSB = nc.declare_dram_parameter("ws_b", [P, 1], FP, False)
    ONES = nc.declare_dram_parameter("ones49", [1, P], BF, False)
    EYE = nc.declare_dram_parameter("eye128", [128, 128], FP, False)
    OUT = nc.declare_dram_parameter("out", [bloc, 2 * H + P + 1], FP, True)

    with tile.TileContext(nc) as tc, ExitStack() as ctx:
        singles = ctx.enter_context(tc.tile_pool(name="singles", bufs=1))

        wvT = singles.tile([128, HC, P], BF, tag="wvT")
        nc.sync.dma_start(out=wvT, in_=WVT[:, :, :])
        wgT = singles.tile([128, HC, P], BF, tag="wgT")
        nc.sync.dma_start(out=wgT, in_=WGT[:, :, :])
        wsT = singles.tile([128, HC, P], BF, tag="wsT")
        nc.sync.dma_start(out=wsT, in_=WST[:, :, :])
        whT = singles.tile([P, 1], BF, tag="whT")
        nc.sync.dma_start(out=whT, in_=WHT[:, :])
        wvb = singles.tile([P, 1], FP, tag="wvb")
        nc.sync.dma_start(out=wvb, in_=WVB[:, :])
        wsb = singles.tile([P, 1], FP, tag="wsb")
        nc.sync.dma_start(out=wsb, in_=WSB[:, :])
        ones = singles.tile([1, P], BF, tag="ones")
        nc.sync.dma_start(out=ones, in_=ONES[:, :])
        eye = singles.tile([128, 128], FP, tag="eye")
        nc.sync.dma_start(out=eye, in_=EYE[:, :])
        eprm = singles.tile([P, NCH, 128], BF, tag="eprm")
        nc.sync.dma_start(out=eprm, in_=EPRM[:, :, :])
        bmsk = singles.tile([128, NCH, HALF], BF, tag="bmsk")
        nc.sync.dma_start(out=bmsk, in_=BMSK[:, :, :])

        avs_row = singles.tile([1, bloc], FP, tag="avs")

        # ---- phase A: htp/stp/avs, htp rows in (b,p) order ----
        with (
            tc.tile_pool(name="pa_sb", bufs=1) as pa,
            tc.tile_pool(name="pa_ps", bufs=2, space="PSUM") as pap,
        ):
            htT = pa.tile([128, HC, bloc], BF, tag="htT")
            nc.sync.dma_start(out=htT, in_=HTT[:, :, :])
            stT = pa.tile([128, HC, bloc], BF, tag="stT")
            nc.sync.dma_start(out=stT, in_=STT[:, :, :])

            ps_htp = pap.tile([P, bloc], FP, tag="p")
            for hc in range(HC):
                nc.tensor.matmul(ps_htp, wgT[:, hc, :], htT[:, hc, :],
                                 start=(hc == 0), stop=(hc == HC - 1))
            htp = pa.tile([P, bloc], FP, tag="htp")
            nc.vector.tensor_copy(out=htp, in_=ps_htp)

            ps_stp = pap.tile([P, bloc], FP, tag="p")
            for hc in range(HC):
                nc.tensor.matmul(ps_stp, wsT[:, hc, :], stT[:, hc, :],
                                 start=(hc == 0), stop=(hc == HC - 1))
            sh = pa.tile([P, bloc], FP, tag="sh")
            nc.vector.tensor_add(out=sh, in0=htp, in1=ps_stp)
            avt = pa.tile([P, bloc], BF, tag="avt")
            nc.scalar.activation(out=avt, in_=sh,
                                 func=mybir.ActivationFunctionType.Tanh,
                                 bias=wsb, scale=1.0)
            ps_avs = pap.tile([1, bloc], FP, tag="pavs")
            nc.tensor.matmul(ps_avs, whT, avt, start=True, stop=True)
            nc.vector.tensor_copy(out=avs_row, in_=ps_avs)

            # htpT (b-major, bf16) kept resident for per-half row extraction
            g128 = min(128, bloc)
            ngr = bloc // g128
            htpT = singles.tile([g128, ngr, P], BF, tag="htpT")
            for g in range(ngr):
                ps_t = pap.tile([g128, P], FP, tag="pt")
                nc.tensor.transpose(ps_t, htp[:, g * g128:(g + 1) * g128],
                                    eye[0:P, 0:P])
                nc.vector.tensor_copy(out=htpT[:, g, :], in_=ps_t)

        # ---- main: fine-grained software pipeline over halves ----
        # per half i: img/tanh/z btiles (pass1) run while the previous
        # half's softmax->S->context matmuls (pass2) interleave btile-wise.
        vimg = ctx.enter_context(tc.tile_pool(name="vimg", bufs=3))
        vctxp = ctx.enter_context(tc.tile_pool(name="vctx", bufs=6))
        thp = ctx.enter_context(tc.tile_pool(name="th", bufs=3))
        smp = ctx.enter_context(tc.tile_pool(name="sm", bufs=2))
        outp = ctx.enter_context(tc.tile_pool(name="outp", bufs=2))
        pimg = ctx.enter_context(tc.tile_pool(name="pimg", bufs=2, space="PSUM"))
        pz = ctx.enter_context(tc.tile_pool(name="pz", bufs=2, space="PSUM"))
        pt = ctx.enter_context(tc.tile_pool(name="pt", bufs=2, space="PSUM"))
        pctx = ctx.enter_context(tc.tile_pool(name="pctx", bufs=2, space="PSUM"))

        g128 = min(128, bloc)
        HB = HALF * P // 2          # free size of one A/B img tile (4 btiles)
        vi_tiles = {}               # (i, 0|1) -> tile
        vc_tiles = {}               # (i, hc) -> tile
        sm_st = {}                  # i -> dict of softmax tiles

        def emit_vi(i, j):
            vt = vimg.tile([128, HC, HB], BF, tag="vi", name=f"vi{i}_{j}")
            nc.sync.dma_start(out=vt, in_=V_IMG[i, j])
            vi_tiles[(i, j)] = vt

        def emit_rows(i):
            rows_h = smp.tile([1, HALF * P], BF, tag="rows")
            pb = (i * HALF) % g128
            nc.gpsimd.dma_start(
                out=rows_h.rearrange("o (b q) -> o b q", b=HALF, q=P),
                in_=htpT[pb:pb + HALF, (i * HALF) // g128, :])
            sm_st[i] = {"rows": rows_h}

        def emit_vc(i, hc):
            vc = vctxp.tile([128, NCH, 128], BF, tag="vc",
                            name=f"vc{i}_{hc}")
            nc.sync.dma_start(out=vc, in_=V_CTX[hc, i])
            vc_tiles[(i, hc)] = vc

        def open_z50(i):
            z50 = smp.tile([HALF, P + 1], FP, tag="z50")
            nc.gpsimd.dma_start(out=z50[:, P:P + 1],
                                in_=avs_row[0:1, i * HALF:(i + 1) * HALF])
            sm_st[i]["z50"] = z50

        def img_btile(i, t):
            st = sm_st[i]
            vt = vi_tiles[(i, t // 4)]
            tf = (t % 4) * FREE
            ps_img = pimg.tile([P, FREE], FP, tag="img")
            for hc in range(HC):
                nc.tensor.matmul(ps_img, wvT[:, hc, :],
                                 vt[:, hc, tf:tf + FREE],
                                 start=(hc == 0), stop=False)
            nc.tensor.matmul(ps_img, ones,
                             st["rows"][0:1, t * FREE:(t + 1) * FREE],
                             start=False, stop=True)
            th = thp.tile([P, FREE], BF, tag="th")
            nc.scalar.activation(out=th, in_=ps_img,
                                 func=mybir.ActivationFunctionType.Tanh,
                                 bias=wvb, scale=1.0)
            ps_z = pz.tile([1, FREE], FP, tag="z")
            nc.tensor.matmul(ps_z, whT, th, start=True, stop=True)
            zrow = thp.tile([1, FREE], FP, tag="zrow")
            nc.vector.tensor_copy(out=zrow, in_=ps_z)
            nc.gpsimd.dma_start(
                out=st["z50"][t * BT:(t + 1) * BT, 0:P],
                in_=zrow.rearrange("o (bi p) -> o bi p", bi=BT, p=P))
            if t == 3:
                vi_tiles.pop((i, 0))
            elif t == 7:
                vi_tiles.pop((i, 1))

        def softmax_dve(i):
            st = sm_st[i]
            z50 = st["z50"]
            m = smp.tile([HALF, 1], FP, tag="m")
            nc.vector.reduce_max(out=m, in_=z50, axis=mybir.AxisListType.X)
            mneg = smp.tile([HALF, 1], FP, tag="mneg")
            nc.scalar.mul(out=mneg, in_=m, mul=-1.0)
            e = smp.tile([HALF, P + 1], FP, tag="e")
            nc.scalar.activation(out=e, in_=z50,
                                 func=mybir.ActivationFunctionType.Exp,
                                 bias=mneg, scale=1.0)
            s49 = smp.tile([HALF, 1], FP, tag="s49")
            nc.vector.reduce_sum(out=s49, in_=e[:, 0:P],
                                 axis=mybir.AxisListType.X)
            s50 = smp.tile([HALF, 1], FP, tag="s50")
            nc.vector.tensor_add(out=s50, in0=s49, in1=e[:, P:P + 1])
            nc.vector.reciprocal(out=s49, in_=s49)
            nc.vector.reciprocal(out=s50, in_=s50)
            alpha = smp.tile([HALF, P], FP, tag="alpha")
            nc.vector.tensor_scalar_mul(out=alpha, in0=e[:, 0:P], scalar1=s49)
            beta = smp.tile([HALF, 1], FP, tag="beta")
            nc.vector.tensor_mul(out=beta, in0=e[:, P:P + 1], in1=s50)
            st["alpha"], st["beta"] = alpha, beta

        def emit_aT_S(i):
            st = sm_st[i]
            ps_at = pt.tile([P, HALF], FP, tag="pt")
            nc.tensor.transpose(ps_at, st["alpha"], eye[0:HALF, 0:HALF])
            alphaT = smp.tile([P, HALF], BF, tag="alphaT")
            nc.vector.tensor_copy(out=alphaT, in_=ps_at)
            D = smp.tile([128, NCH, HALF], BF, tag="D")
            for c in range(NCH):
                ps_d = pt.tile([128, HALF], FP, tag="pt", name="psd")
                nc.tensor.matmul(ps_d, eprm[:, c, :], alphaT,
                                 start=True, stop=True)
                nc.vector.tensor_mul(out=D[:, c, :], in0=ps_d,
                                     in1=bmsk[:, c, :])
            st["D"] = D
            cc = outp.tile([HALF, 2 * H], FP, tag="cc", name="cc")
            st["cc"] = cc
            st["ctx_sb"] = cc[:, H:2 * H]

        def ctx_hc(i, hc):
            st = sm_st[i]
            D = st["D"]
            vc = vc_tiles.pop((i, hc))
            ps_hc = pctx.tile([128, HALF], FP, tag="ctx")
            for c in range(NCH):
                nc.tensor.matmul(ps_hc, vc[:, c, :], D[:, c, :],
                                 start=(c == 0), stop=(c == NCH - 1))
            ctxh = thp.tile([128, HALF], FP, tag="ctxh")
            nc.vector.tensor_copy(out=ctxh, in_=ps_hc)
            ps_ct = pt.tile([HALF, 128], FP, tag="pt")
            nc.tensor.transpose(ps_ct, ctxh, eye)
            nc.vector.tensor_copy(
                out=st["ctx_sb"][:, hc * 128:(hc + 1) * 128], in_=ps_ct)

        def blend_out(i):
            st = sm_st.pop(i)
            ctx_sb, alpha, beta = st["ctx_sb"], st["alpha"], st["beta"]
            stn = outp.tile([HALF, H], FP, tag="stn")
            nc.scalar.dma_start(out=stn, in_=STN[i * HALF:(i + 1) * HALF, :])
            nc.vector.tensor_sub(out=stn, in0=stn, in1=ctx_sb)
            nc.vector.tensor_scalar_mul(out=stn, in0=stn, scalar1=beta)
            cc = st["cc"]
            nc.vector.tensor_add(out=cc[:, 0:H], in0=stn, in1=ctx_sb)
            sl = slice(i * HALF, (i + 1) * HALF)
            nc.scalar.dma_start(out=OUT[sl, 0:2 * H], in_=cc)
            nc.gpsimd.dma_start(out=OUT[sl, 2 * H:2 * H + P], in_=alpha)
            nc.gpsimd.dma_start(out=OUT[sl, 2 * H + P:2 * H + P + 1],
                                in_=beta)

        emit_vi(0, 0)
        emit_vi(0, 1)
        emit_rows(0)
        for i in range(nhalf):
            open_z50(i)
            if i > 0:
                softmax_dve(i - 1)
                emit_vc(i - 1, 0)
                emit_vc(i - 1, 1)
            if i + 1 < nhalf:
                emit_vi(i + 1, 0)
                emit_rows(i + 1)
            for t in range(BT):
                if i > 0 and t + 2 < HC:
                    emit_vc(i - 1, t + 2)
                if t == 4 and i + 1 < nhalf:
                    emit_vi(i + 1, 1)
                img_btile(i, t)
                if i > 0 and t == 0:
                    emit_aT_S(i - 1)
                if i > 0 and t >= 1:
                    ctx_hc(i - 1, t - 1)
            if i > 0:
                ctx_hc(i - 1, HC - 1)
                blend_out(i - 1)
        # tail: last half's softmax + context
        last = nhalf - 1
        softmax_dve(last)
        emit_aT_S(last)
        for hc in range(HC):
            emit_vc(last, hc)
            ctx_hc(last, hc)
        blend_out(last)

    nc.finalize()
    return nc


def prep_inputs_core(Vc, htc, stc, w):
    """Per-core input map. Vc (bloc,H,P) f32, htc/stc (bloc,H) f32."""
    bloc = Vc.shape[0]
    nhalf = bloc // HALF
    # v_img[i, j, h, hc, (bq p)] = V[i*64+j*32+bq, hc*128+h, p]
    v_img = np.ascontiguousarray(
        Vc.reshape(nhalf, 2, 32, HC, 128, P).transpose(0, 1, 4, 3, 2, 5)
        .reshape(nhalf, 2, 128, HC, HALF * P // 2)
    ).astype(BF16)
    # v_ctx[hc, half, c, r, h] = V[half*64 + (128c+r)//49, hc*128+h, (128c+r)%49]
    vt = (Vc.reshape(nhalf, HALF, HC, 128, P).transpose(2, 0, 1, 4, 3)
          .reshape(HC, nhalf, HALF * P, 128))
    v_ctx = np.zeros((HC, nhalf, NCH * 128, 128), np.float32)
    v_ctx[:, :, :HALF * P, :] = vt
    v_ctx = np.ascontiguousarray(
        v_ctx.reshape(HC, nhalf, NCH, 128, 128).transpose(0, 1, 3, 2, 4)
    ).astype(BF16)
    htT = np.ascontiguousarray(
        htc.reshape(bloc, HC, 128).transpose(2, 1, 0)).astype(BF16)
    stT = np.ascontiguousarray(
        stc.reshape(bloc, HC, 128).transpose(2, 1, 0)).astype(BF16)
    m = {
        "v_img": v_img, "v_ctx": v_ctx, "htT": htT, "stT": stT,
        "stn": np.ascontiguousarray(stc, dtype=np.float32),
    }
    m.update(w)
    return m


def prep_weights(Wv_w, Wv_b, Ws_w, Ws_b, Wg_w, wh_w):
    def wT(W):  # (P, H) -> (128, HC, P)
        return np.ascontiguousarray(
            W.reshape(P, HC, 128).transpose(2, 1, 0)).astype(BF16)
    return {
        "wvT": wT(Wv_w), "wgT": wT(Wg_w), "wsT": wT(Ws_w),
        "whT": np.ascontiguousarray(wh_w.reshape(P, 1)).astype(BF16),
        "wv_b": np.ascontiguousarray(Wv_b.reshape(P, 1), dtype=np.float32),
        "ws_b": np.ascontiguousarray(Ws_b.reshape(P, 1), dtype=np.float32),
        "ones49": np.ones((1, P), dtype=BF16),
        "eye128": np.eye(128, dtype=np.float32),
        "eperm": _eperm(), "bmask": _bmask(),
    }


def _eperm():
    e = np.zeros((P, NCH, 128), np.float32)
    r = np.arange(NCH * 128)
    valid = r < HALF * P
    e[(r % P)[valid], (r // 128)[valid], (r % 128)[valid]] = 1.0
    return e.astype(BF16)


def _bmask():
    b = np.zeros((128, NCH, HALF), np.float32)
    r = np.arange(NCH * 128)
    valid = r < HALF * P
    b[(r % 128)[valid], (r // 128)[valid], (r // P)[valid]] = 1.0
    return b.astype(BF16)


_nc_cache = {}


def _get_nc(nhalf):
    if nhalf not in _nc_cache:
        _nc_cache[nhalf] = build_nc(nhalf)
    return _nc_cache[nhalf]


def kernel(V, ht, st, Wv_w, Wv_b, Ws_w, Ws_b, Wg_w, wh_w, **_run_kw):
    V = np.asarray(V, dtype=np.float32)
    ht = np.asarray(ht, dtype=np.float32)
    st = np.asarray(st, dtype=np.float32)
    w = prep_weights(np.asarray(Wv_w, np.float32), np.asarray(Wv_b, np.float32),
                     np.asarray(Ws_w, np.float32), np.asarray(Ws_b, np.float32),
                     np.asarray(Wg_w, np.float32), np.asarray(wh_w, np.float32))
    nc = _get_nc(BLOC // HALF)
    in_maps = []
    for c in range(NCORES):
        sl = slice(c * BLOC, (c + 1) * BLOC)
        in_maps.append(prep_inputs_core(V[sl], ht[sl], st[sl], w))
    res = run_bass_kernel_spmd(nc, in_maps, core_ids=list(range(NCORES)), **_run_kw)
    outs = [res.results[c]["out"] for c in range(NCORES)]
    full = np.concatenate(outs, axis=0)
    c_t = full[:, 0:H]
    ctxo = full[:, H:2 * H]
    alpha = full[:, 2 * H:2 * H + P]
    beta = full[:, 2 * H + P:2 * H + P + 1]
    kernel.last_result = res
    return (c_t, ctxo, alpha, beta)
